# revision 13
# baseline (speedup 1.0000x reference)
"""BiDAF attention-flow kernel for one TRN2 chip (8 NeuronCores).

Reference computation (per batch b):
    w1, w2, w3 = w[:D], w[D:2D], w[2D:]
    sim[c,q] = w1.C_c + w2.Q_q + w3.(C_c*Q_q)          # trilinear similarity
    c2q = softmax_q(sim) @ Q                            # [Lc, D]
    batt = softmax_c(max_q sim)                         # [Lc]
    q2c  = batt @ C, broadcast over Lc                  # [Lc, D]
    returns (c2q, q2c_broadcast)

Sharding: pure data parallel — batch 32 split 4-per-core over 8 cores, w
replicated.  No collectives.

Device algorithm per core (4 batches), bf16 matmul inputs / f32 PSUM:
  - C, Q loaded with f32->bf16 cast during SWDGE DMA.
  - sim kept in [q, c] layout: lhsT = (w3*Q)^T tile (stationary, reused
    across the c stream), rhs = C^T streamed N=512.  PE transposes C/Q
    (bf16, grouped 8 per one [128,1024]-bf16 PSUM bank).
  - s1[c] = C@w1 and s2[q] = Q@w2 via DVE fused mul+reduce against
    partition-broadcast copies of w1/w2 (results land as columns).
  - softmax over q never needs s1 (constant per column in this layout):
    ET = exp(sim + s2) via ACT with per-partition bias; column sums/max of
    ET over q come from PE-transposed ET tiles reduced straight out of
    PSUM.
  - c2q = (ET/rsum)^T @ Q with ET used directly as lhsT; 1/rsum applied
    per-partition during PSUM evacuation.
  - z = max_q(ET) * exp(s1)  (exp is monotonic), q2c = (z @ C)/sum(z).
q2c is returned [B, D] and broadcast to [B, Lc, D] on host (replication =
part of unsharding).
"""

import sys

for _p in ("/opt/trn_rl_repo", "/root/.axon_site/_ro/trn_rl_repo"):
    if _p not in sys.path:
        sys.path.append(_p)

from contextlib import ExitStack

import numpy as np

import concourse.bacc as bacc
import concourse.bass as bass
import concourse.tile as tile
from concourse import mybir
from concourse.bass_utils import run_bass_kernel_spmd
from concourse.masks import make_identity

if __import__("os").environ.get("BASS_LDW_OPT") == "1":
    import concourse.bass_utils as _bu

    _orig_run_command = _bu.run_command

    def _patched_run_command(argv, **kw):
        argv = [
            "--enable-ldw-opt=true" if a == "--enable-ldw-opt=false" else a
            for a in argv
        ]
        return _orig_run_command(argv, **kw)

    _bu.run_command = _patched_run_command

F32 = mybir.dt.float32
BF16 = mybir.dt.bfloat16
AF = mybir.ActivationFunctionType
AX = mybir.AxisListType
ALU = mybir.AluOpType

B, LC, LQ, D = 32, 1024, 128, 1024
NCORES = 8
BPC = B // NCORES  # batches per core
NCT = LC // 128  # c-tiles per batch
NDT = D // 128  # d-tiles

_NC_CACHE = None


def build_kernel():
    nc = bacc.Bacc("TRN2", target_bir_lowering=False, debug=False, num_devices=NCORES)
    ctx_ext = nc.dram_tensor("ctx", [BPC, LC, D], F32, kind="ExternalInput").ap()
    q_ext = nc.dram_tensor("q", [BPC, LQ, D], F32, kind="ExternalInput").ap()
    w_ext = nc.dram_tensor("w", [3 * D], F32, kind="ExternalInput").ap()
    c2q_ext = nc.dram_tensor("c2q", [BPC, LC, D], F32, kind="ExternalOutput").ap()
    q2c_ext = nc.dram_tensor("q2c", [BPC, D], F32, kind="ExternalOutput").ap()

    with tile.TileContext(nc) as tc, ExitStack() as ctx:
        consts = ctx.enter_context(tc.tile_pool(name="consts", bufs=1))
        cn_pool = ctx.enter_context(tc.tile_pool(name="cn", bufs=2 * NCT))
        ct_pool = ctx.enter_context(tc.tile_pool(name="ct", bufs=2 * NDT))
        qn_pool = ctx.enter_context(tc.tile_pool(name="qn", bufs=2))
        et_pool = ctx.enter_context(tc.tile_pool(name="et", bufs=4))
        out_pool = ctx.enter_context(tc.tile_pool(name="outs", bufs=4))
        small = ctx.enter_context(tc.tile_pool(name="small", bufs=4))
        # PSUM: 8 banks.  tags: tpose(2) + simp(2) + c2qp(2) + q2cp(2)
        tp_psum = ctx.enter_context(tc.tile_pool(name="tpose", bufs=2, space="PSUM"))
        sim_psum = ctx.enter_context(tc.tile_pool(name="simp", bufs=2, space="PSUM"))
        c2q_psum = ctx.enter_context(tc.tile_pool(name="c2qp", bufs=2, space="PSUM"))
        q2c_psum = ctx.enter_context(tc.tile_pool(name="q2cp", bufs=2, space="PSUM"))

        # ---- constants ----
        ident_bf = consts.tile([128, 128], BF16)
        make_identity(nc, ident_bf)
        ident_f32 = consts.tile([128, 128], F32)
        make_identity(nc, ident_f32)
        ones_col = consts.tile([128, 1], BF16)
        nc.vector.memset(ones_col, 1.0)
        ones_row = consts.tile([1, 128], BF16)
        nc.vector.memset(ones_row, 1.0)

        # w1, w2, w3 as per-partition columns [128, NDT]
        wsb = [
            consts.tile([NDT, 128], F32, tag=f"wsb{i}", name=f"wsb{i}")
            for i in range(3)
        ]
        for i in range(3):
            nc.sync.dma_start(
                out=wsb[i],
                in_=w_ext[i * D : (i + 1) * D].rearrange("(a b) -> a b", b=128),
            )
        wp = []
        for i in range(3):
            p = tp_psum.tile([128, NDT], F32, tag="tpose", name=f"wp{i}")
            nc.tensor.transpose(p, wsb[i], ident_f32[:NDT, :NDT])
            wp.append(p)
        w1cols = consts.tile([128, NDT], BF16)
        nc.vector.tensor_copy(w1cols, wp[0])
        w2cols = consts.tile([128, NDT], F32)
        nc.vector.tensor_copy(w2cols, wp[1])
        w3cols = consts.tile([128, NDT], F32)
        nc.vector.tensor_copy(w3cols, wp[2])
        w3inv = consts.tile([128, NDT], F32)
        nc.vector.reciprocal(w3inv, w3cols)
        w23cols = consts.tile([128, NDT], BF16)
        nc.vector.tensor_mul(w23cols, w2cols, w3inv)

        evac = 0  # DVE/ACT alternation counter

        for b in range(BPC):
            # ---- loads (cast f32 -> bf16 during DMA) ----
            qn = qn_pool.tile([LQ, D], BF16, tag="qn")
            nc.gpsimd.dma_start(out=qn, in_=q_ext[b])
            cn = []
            for ci in range(NCT):
                t = cn_pool.tile([128, D], BF16, tag="cn", name=f"cn{b}_{ci}")
                nc.gpsimd.dma_start(out=t, in_=ctx_ext[b, ci * 128 : (ci + 1) * 128])
                cn.append(t)

            # ---- Q transpose, scaled by w3:  qt3[d, q] = w3[d] * Q[q, d]^T ----
            qtp = tp_psum.tile([128, D], BF16, tag="tpose")
            for dt in range(NDT):
                nc.tensor.transpose(
                    qtp[:, dt * 128 : (dt + 1) * 128],
                    qn[:, dt * 128 : (dt + 1) * 128],
                    ident_bf,
                )
            qt3 = qn_pool.tile([128, D], BF16, tag="qt3")
            for dt in range(NDT):
                nc.vector.tensor_scalar_mul(
                    qt3[:, dt * 128 : (dt + 1) * 128],
                    qtp[:, dt * 128 : (dt + 1) * 128],
                    w3cols[:, dt : dt + 1],
                )

            # ---- C transpose interleaved with sim/s1/s2 matmuls so the PE
            # alternates transpose and matmul work (keeps HAM warm) ----
            ctb = []
            simp = []
            for g in range(2):
                sp = sim_psum.tile([128, 512], F32, tag="simp", name=f"simp{b}_{g}")
                simp.append(sp)
            s2p = q2c_psum.tile([128, 1], F32, tag="q2cp", name=f"s2p{b}")
            # s1row chunks packed into PSUM column groups 0/32: the two M=1
            # matmuls run concurrently on the PE via tile_position
            s1rp = q2c_psum.tile([64, 512], F32, tag="q2cp", name=f"s1rp{b}")
            for dt in range(NDT):
                ctp = tp_psum.tile([128, LC], BF16, tag="tpose", name=f"ctp{b}_{dt}")
                for ci in range(NCT):
                    nc.tensor.transpose(
                        ctp[:, ci * 128 : (ci + 1) * 128],
                        cn[ci][:, dt * 128 : (dt + 1) * 128],
                        ident_bf,
                    )
                t = ct_pool.tile([128, LC], BF16, tag="ct", name=f"ct{b}_{dt}")
                if evac % 2 == 0:
                    nc.vector.tensor_copy(t, ctp)
                else:
                    nc.scalar.copy(t, ctp)
                evac += 1
                ctb.append(t)
                for g in range(2):
                    nc.tensor.matmul(
                        simp[g],
                        qt3[:, dt * 128 : (dt + 1) * 128],
                        ctb[dt][:, g * 512 : (g + 1) * 512],
                        start=(dt == 0),
                        stop=False,
                    )
                # s2[q] column = sum_d (w2/w3)[d] * qt3[d, q], same weights
                nc.tensor.matmul(
                    s2p,
                    qt3[:, dt * 128 : (dt + 1) * 128],
                    w23cols[:, dt : dt + 1],
                    start=(dt == 0),
                    stop=(dt == NDT - 1),
                )
                for g in range(2):
                    nc.tensor.matmul(
                        s1rp[32 * g : 32 * g + 1, :],
                        w1cols[:, dt : dt + 1],
                        ctb[dt][:, g * 512 : (g + 1) * 512],
                        start=(dt == 0),
                        stop=(dt == NDT - 1),
                        tile_position=(0, 32 * g),
                    )
            s2c = small.tile([128, 1], F32, tag="s2c")
            nc.vector.tensor_copy(s2c, s2p)

            # fold s1[c] into sim as the last accumulation step (a K=1
            # matmul adding s1row to every q row).  softmax over q is
            # invariant to it, and max_q then includes s1 so z = max directly.
            s1row_sb = small.tile([1, LC], BF16, tag="s1row")
            for g in range(2):
                nc.vector.tensor_copy(
                    s1row_sb[:, g * 512 : (g + 1) * 512],
                    s1rp[32 * g : 32 * g + 1, :],
                )
            for g in range(2):
                nc.tensor.matmul(
                    simp[g],
                    ones_row,
                    s1row_sb[0:1, g * 512 : (g + 1) * 512],
                    start=False,
                    stop=True,
                )

            # ---- ET = exp(sim + s2)  [q, c] bf16 ----
            et = []
            for g in range(2):
                e = et_pool.tile([128, 512], BF16, tag="et", name=f"et{b}_{g}")
                nc.scalar.activation(e, simp[g], AF.Exp, bias=s2c)
                et.append(e)

            # ---- ET transposed -> column-wise max (z) and sum (rsum).
            # One fast evac to SBUF releases the PSUM bank early; the max and
            # sum then run as single 3D-AP reduces. ----
            rsums = small.tile([128, NCT], F32, tag="rsums")
            etp = tp_psum.tile([128, LC], BF16, tag="tpose", name=f"etp{b}")
            for ci in range(NCT):
                nc.tensor.transpose(
                    etp[:, ci * 128 : (ci + 1) * 128],
                    et[ci // 4][:, (ci % 4) * 128 : (ci % 4 + 1) * 128],
                    ident_bf,
                )
            ets = qn_pool.tile([128, LC], BF16, tag="ets", name=f"ets{b}")
            nc.vector.tensor_copy(ets, etp)
            ets3 = ets.rearrange("p (t c) -> p t c", c=128)
            zcols = small.tile([128, NCT], BF16, tag="zcols")
            nc.vector.reduce_max(zcols, ets3, axis=AX.X)
            nc.vector.reduce_sum(rsums, ets3, axis=AX.X)

            rinvs = small.tile([128, NCT], F32, tag="rinvs")
            nc.vector.reciprocal(rinvs, rsums)

            # ---- c2q matmuls per c-tile ----
            for ci in range(NCT):
                lhs = et[ci // 4][:, (ci % 4) * 128 : (ci % 4 + 1) * 128]
                c2q_sb = out_pool.tile(
                    [128, D], F32, tag="c2q_sb", name=f"c2qsb{b}_{ci}"
                )
                for ch in range(2):
                    cp = c2q_psum.tile(
                        [128, 512], F32, tag="c2qp", name=f"cp{b}_{ci}_{ch}"
                    )
                    nc.tensor.matmul(
                        cp,
                        lhs,
                        qn[:, ch * 512 : (ch + 1) * 512],
                        start=True,
                        stop=True,
                    )
                    if ch == 0:
                        nc.vector.tensor_scalar_mul(
                            c2q_sb[:, ch * 512 : (ch + 1) * 512],
                            cp,
                            rinvs[:, ci : ci + 1],
                        )
                    else:
                        nc.scalar.mul(
                            c2q_sb[:, ch * 512 : (ch + 1) * 512],
                            cp,
                            rinvs[:, ci : ci + 1],
                        )
                nc.sync.dma_start(
                    out=c2q_ext[b, ci * 128 : (ci + 1) * 128], in_=c2q_sb
                )

            # ---- q2c = (z @ C) / sum(z), chunks packed in col groups ----
            q2cpp = q2c_psum.tile([64, 512], F32, tag="q2cp", name=f"q2cpp{b}")
            for ci in range(NCT):
                for ch in range(2):
                    nc.tensor.matmul(
                        q2cpp[32 * ch : 32 * ch + 1, :],
                        zcols[:, ci : ci + 1],
                        cn[ci][:, ch * 512 : (ch + 1) * 512],
                        start=(ci == 0),
                        stop=(ci == NCT - 1),
                        tile_position=(0, 32 * ch),
                    )
            q2cp = [q2cpp[0:1, :], q2cpp[32:33, :]]
            zsp = tp_psum.tile([1, NCT], F32, tag="tpose", name=f"zsp{b}")
            nc.tensor.matmul(zsp, ones_col, zcols, start=True, stop=True)
            zsum = small.tile([1, 1], F32, tag="zsum")
            nc.vector.reduce_sum(zsum, zsp, axis=AX.X)
            zrinv = small.tile([1, 1], F32, tag="zrinv")
            nc.vector.reciprocal(zrinv, zsum)
            q2c_sb = out_pool.tile([1, D], F32, tag="q2c_sb", name=f"q2csb{b}")
            for ch in range(2):
                nc.vector.tensor_scalar_mul(
                    q2c_sb[:, ch * 512 : (ch + 1) * 512], q2cp[ch], zrinv
                )
            nc.sync.dma_start(out=q2c_ext[b : b + 1, :], in_=q2c_sb)

    nc.compile()
    return nc


def _get_nc():
    global _NC_CACHE
    if _NC_CACHE is None:
        _NC_CACHE = build_kernel()
    return _NC_CACHE


def kernel(context_features, question_features, w, _trace=False):
    nc = _get_nc()
    context_features = np.ascontiguousarray(context_features, dtype=np.float32)
    question_features = np.ascontiguousarray(question_features, dtype=np.float32)
    w = np.ascontiguousarray(w, dtype=np.float32)
    in_maps = []
    for core in range(NCORES):
        b0 = core * BPC
        in_maps.append(
            {
                "ctx": context_features[b0 : b0 + BPC],
                "q": question_features[b0 : b0 + BPC],
                "w": w,
            }
        )
    res = run_bass_kernel_spmd(
        nc, in_maps, core_ids=list(range(NCORES)), trace=_trace
    )
    c2q = np.concatenate([res.results[i]["c2q"] for i in range(NCORES)], axis=0)
    q2c_vec = np.concatenate([res.results[i]["q2c"] for i in range(NCORES)], axis=0)
    q2c = np.broadcast_to(q2c_vec[:, None, :], (B, LC, D))
    if _trace:
        kernel.last_exec_time_ns = res.exec_time_ns
    return (c2q, q2c)


# revision 14
# speedup vs baseline: 1.0480x; 1.0480x over previous
"""BiDAF attention-flow kernel for one TRN2 chip (8 NeuronCores).

Reference computation (per batch b):
    w1, w2, w3 = w[:D], w[D:2D], w[2D:]
    sim[c,q] = w1.C_c + w2.Q_q + w3.(C_c*Q_q)          # trilinear similarity
    c2q = softmax_q(sim) @ Q                            # [Lc, D]
    batt = softmax_c(max_q sim)                         # [Lc]
    q2c  = batt @ C, broadcast over Lc                  # [Lc, D]
    returns (c2q, q2c_broadcast)

Sharding: pure data parallel — batch 32 split 4-per-core over 8 cores, w
replicated.  No collectives.

Device algorithm per core (4 batches), bf16 matmul inputs / f32 PSUM:
  - C, Q loaded with f32->bf16 cast during SWDGE DMA.
  - sim kept in [q, c] layout: lhsT = (w3*Q)^T tile (stationary, reused
    across the c stream), rhs = C^T streamed N=512.  PE transposes C/Q
    (bf16, grouped 8 per one [128,1024]-bf16 PSUM bank).
  - s1[c] = C@w1 and s2[q] = Q@w2 via DVE fused mul+reduce against
    partition-broadcast copies of w1/w2 (results land as columns).
  - softmax over q never needs s1 (constant per column in this layout):
    ET = exp(sim + s2) via ACT with per-partition bias; column sums/max of
    ET over q come from PE-transposed ET tiles reduced straight out of
    PSUM.
  - c2q = (ET/rsum)^T @ Q with ET used directly as lhsT; 1/rsum applied
    per-partition during PSUM evacuation.
  - z = max_q(ET) * exp(s1)  (exp is monotonic), q2c = (z @ C)/sum(z).
q2c is returned [B, D] and broadcast to [B, Lc, D] on host (replication =
part of unsharding).
"""

import sys

for _p in ("/opt/trn_rl_repo", "/root/.axon_site/_ro/trn_rl_repo"):
    if _p not in sys.path:
        sys.path.append(_p)

from contextlib import ExitStack

import numpy as np

import concourse.bacc as bacc
import concourse.bass as bass
import concourse.tile as tile
from concourse import mybir
from concourse.bass_utils import run_bass_kernel_spmd
from concourse.masks import make_identity

if __import__("os").environ.get("BASS_LDW_OPT") == "1":
    import concourse.bass_utils as _bu

    _orig_run_command = _bu.run_command

    def _patched_run_command(argv, **kw):
        argv = [
            "--enable-ldw-opt=true" if a == "--enable-ldw-opt=false" else a
            for a in argv
        ]
        return _orig_run_command(argv, **kw)

    _bu.run_command = _patched_run_command

F32 = mybir.dt.float32
BF16 = mybir.dt.bfloat16
AF = mybir.ActivationFunctionType
AX = mybir.AxisListType
ALU = mybir.AluOpType

B, LC, LQ, D = 32, 1024, 128, 1024
NCORES = 8
BPC = B // NCORES  # batches per core
NCT = LC // 128  # c-tiles per batch
NDT = D // 128  # d-tiles

_NC_CACHE = None


def build_kernel():
    nc = bacc.Bacc("TRN2", target_bir_lowering=False, debug=False, num_devices=NCORES)
    ctx_ext = nc.dram_tensor("ctx", [BPC, LC, D], F32, kind="ExternalInput").ap()
    q_ext = nc.dram_tensor("q", [BPC, LQ, D], F32, kind="ExternalInput").ap()
    w_ext = nc.dram_tensor("w", [3 * D], F32, kind="ExternalInput").ap()
    c2q_ext = nc.dram_tensor("c2q", [BPC, LC, D], F32, kind="ExternalOutput").ap()
    q2c_ext = nc.dram_tensor("q2c", [BPC, D], F32, kind="ExternalOutput").ap()

    with tile.TileContext(nc) as tc, ExitStack() as ctx:
        consts = ctx.enter_context(tc.tile_pool(name="consts", bufs=1))
        cn_pool = ctx.enter_context(tc.tile_pool(name="cn", bufs=2 * NCT))
        ct_pool = ctx.enter_context(tc.tile_pool(name="ct", bufs=2 * NDT))
        qn_pool = ctx.enter_context(tc.tile_pool(name="qn", bufs=2))
        et_pool = ctx.enter_context(tc.tile_pool(name="et", bufs=4))
        out_pool = ctx.enter_context(tc.tile_pool(name="outs", bufs=6))
        small = ctx.enter_context(tc.tile_pool(name="small", bufs=4))
        # PSUM: 8 banks.  tags: tpose(2) + simp(2) + c2qp(3) + q2cp(1, sequenced)
        tp_psum = ctx.enter_context(tc.tile_pool(name="tpose", bufs=2, space="PSUM"))
        sim_psum = ctx.enter_context(tc.tile_pool(name="simp", bufs=2, space="PSUM"))
        c2q_psum = ctx.enter_context(tc.tile_pool(name="c2qp", bufs=3, space="PSUM"))
        q2c_psum = ctx.enter_context(tc.tile_pool(name="q2cp", bufs=1, space="PSUM"))

        # ---- constants ----
        ident_bf = consts.tile([128, 128], BF16)
        make_identity(nc, ident_bf)
        ident_f32 = consts.tile([128, 128], F32)
        make_identity(nc, ident_f32)
        ones_col = consts.tile([128, 1], BF16)
        nc.vector.memset(ones_col, 1.0)

        # w1, w2, w3 as per-partition columns [128, NDT]
        wsb = [
            consts.tile([NDT, 128], F32, tag=f"wsb{i}", name=f"wsb{i}")
            for i in range(3)
        ]
        for i in range(3):
            nc.sync.dma_start(
                out=wsb[i],
                in_=w_ext[i * D : (i + 1) * D].rearrange("(a b) -> a b", b=128),
            )
        wp = []
        for i in range(3):
            p = tp_psum.tile([128, NDT], F32, tag="tpose", name=f"wp{i}")
            nc.tensor.transpose(p, wsb[i], ident_f32[:NDT, :NDT])
            wp.append(p)
        w1cols = consts.tile([128, NDT], BF16)
        nc.vector.tensor_copy(w1cols, wp[0])
        w2cols = consts.tile([128, NDT], F32)
        nc.vector.tensor_copy(w2cols, wp[1])
        w3cols = consts.tile([128, NDT], F32)
        nc.vector.tensor_copy(w3cols, wp[2])
        w3inv = consts.tile([128, NDT], F32)
        nc.vector.reciprocal(w3inv, w3cols)
        w23cols = consts.tile([128, NDT], BF16)
        nc.vector.tensor_mul(w23cols, w2cols, w3inv)

        evac = 0  # DVE/ACT alternation counter

        for b in range(BPC):
            # ---- loads (cast f32 -> bf16 during DMA) ----
            qn = qn_pool.tile([LQ, D], BF16, tag="qn")
            nc.gpsimd.dma_start(out=qn, in_=q_ext[b])
            cn = []
            for ci in range(NCT):
                t = cn_pool.tile([128, D], BF16, tag="cn", name=f"cn{b}_{ci}")
                nc.gpsimd.dma_start(out=t, in_=ctx_ext[b, ci * 128 : (ci + 1) * 128])
                cn.append(t)

            # ---- Q transpose, scaled by w3:  qt3[d, q] = w3[d] * Q[q, d]^T ----
            qtp = tp_psum.tile([128, D], BF16, tag="tpose")
            for dt in range(NDT):
                nc.tensor.transpose(
                    qtp[:, dt * 128 : (dt + 1) * 128],
                    qn[:, dt * 128 : (dt + 1) * 128],
                    ident_bf,
                )
            qt3 = qn_pool.tile([128, D], BF16, tag="qt3")
            for dt in range(NDT):
                nc.vector.tensor_scalar_mul(
                    qt3[:, dt * 128 : (dt + 1) * 128],
                    qtp[:, dt * 128 : (dt + 1) * 128],
                    w3cols[:, dt : dt + 1],
                )

            # ---- s2[q] column = sum_d (w2/w3)[d] * qt3[d, q] ----
            s2p = q2c_psum.tile([128, 1], F32, tag="q2cp", name=f"s2p{b}")
            for dt in range(NDT):
                nc.tensor.matmul(
                    s2p,
                    qt3[:, dt * 128 : (dt + 1) * 128],
                    w23cols[:, dt : dt + 1],
                    start=(dt == 0),
                    stop=(dt == NDT - 1),
                )
            s2c = small.tile([128, 1], F32, tag="s2c")
            nc.vector.tensor_copy(s2c, s2p)

            # ---- C transpose interleaved with sim/s1 matmuls so the PE
            # alternates transpose and matmul work (keeps HAM warm) ----
            ctb = []
            simp = []
            for g in range(2):
                sp = sim_psum.tile([128, 512], F32, tag="simp", name=f"simp{b}_{g}")
                simp.append(sp)
            # s1row chunks packed into PSUM column groups 0/32: the two M=1
            # matmuls run concurrently on the PE via tile_position
            s1rp = q2c_psum.tile([64, 512], F32, tag="q2cp", name=f"s1rp{b}")
            for dt in range(NDT):
                ctp = tp_psum.tile([128, LC], BF16, tag="tpose", name=f"ctp{b}_{dt}")
                for ci in range(NCT):
                    nc.tensor.transpose(
                        ctp[:, ci * 128 : (ci + 1) * 128],
                        cn[ci][:, dt * 128 : (dt + 1) * 128],
                        ident_bf,
                    )
                t = ct_pool.tile([128, LC], BF16, tag="ct", name=f"ct{b}_{dt}")
                if evac % 2 == 0:
                    nc.vector.tensor_copy(t, ctp)
                else:
                    nc.scalar.copy(t, ctp)
                evac += 1
                ctb.append(t)
                for g in range(2):
                    nc.tensor.matmul(
                        simp[g],
                        qt3[:, dt * 128 : (dt + 1) * 128],
                        ctb[dt][:, g * 512 : (g + 1) * 512],
                        start=(dt == 0),
                        stop=(dt == NDT - 1),
                    )
                for g in range(2):
                    nc.tensor.matmul(
                        s1rp[32 * g : 32 * g + 1, :],
                        w1cols[:, dt : dt + 1],
                        ctb[dt][:, g * 512 : (g + 1) * 512],
                        start=(dt == 0),
                        stop=(dt == NDT - 1),
                        tile_position=(0, 32 * g),
                    )
            s1row_sb = small.tile([1, LC], F32, tag="s1row")
            for g in range(2):
                nc.vector.tensor_copy(
                    s1row_sb[:, g * 512 : (g + 1) * 512],
                    s1rp[32 * g : 32 * g + 1, :],
                )
            s1p = q2c_psum.tile([128, NCT], F32, tag="q2cp", name=f"s1p{b}")
            for ci in range(NCT):
                nc.tensor.transpose(
                    s1p[:, ci : ci + 1],
                    s1row_sb[0:1, ci * 128 : (ci + 1) * 128],
                    ident_f32[0:1, 0:1],
                )
            s1cols = small.tile([128, NCT], F32, tag="s1cols")
            nc.vector.tensor_copy(s1cols, s1p)
            es1 = small.tile([128, NCT], F32, tag="es1")
            nc.scalar.activation(es1, s1cols, AF.Exp)

            # ---- ET = exp(sim + s2)  [q, c] bf16 ----
            et = []
            for g in range(2):
                e = et_pool.tile([128, 512], BF16, tag="et", name=f"et{b}_{g}")
                nc.scalar.activation(e, simp[g], AF.Exp, bias=s2c)
                et.append(e)

            # ---- ET transposed -> column-wise max (z) and sum (rsum).
            # One fast evac to SBUF releases the PSUM bank early; the max and
            # sum then run as single 3D-AP reduces. ----
            zraw = small.tile([128, NCT], F32, tag="zraw")
            rsums = small.tile([128, NCT], F32, tag="rsums")
            etp = tp_psum.tile([128, LC], BF16, tag="tpose", name=f"etp{b}")
            for ci in range(NCT):
                nc.tensor.transpose(
                    etp[:, ci * 128 : (ci + 1) * 128],
                    et[ci // 4][:, (ci % 4) * 128 : (ci % 4 + 1) * 128],
                    ident_bf,
                )
            ets = qn_pool.tile([128, LC], BF16, tag="ets", name=f"ets{b}")
            nc.vector.tensor_copy(ets, etp)
            ets3 = ets.rearrange("p (t c) -> p t c", c=128)
            nc.vector.reduce_max(zraw, ets3, axis=AX.X)
            nc.vector.reduce_sum(rsums, ets3, axis=AX.X)

            rinvs = small.tile([128, NCT], F32, tag="rinvs")
            nc.vector.reciprocal(rinvs, rsums)
            zcols = small.tile([128, NCT], BF16, tag="zcols")
            nc.vector.tensor_mul(zcols, zraw, es1)

            # ---- c2q matmuls per c-tile ----
            for ci in range(NCT):
                lhs = et[ci // 4][:, (ci % 4) * 128 : (ci % 4 + 1) * 128]
                c2q_sb = out_pool.tile(
                    [128, D], F32, tag="c2q_sb", name=f"c2qsb{b}_{ci}"
                )
                for ch in range(2):
                    cp = c2q_psum.tile(
                        [128, 512], F32, tag="c2qp", name=f"cp{b}_{ci}_{ch}"
                    )
                    nc.tensor.matmul(
                        cp,
                        lhs,
                        qn[:, ch * 512 : (ch + 1) * 512],
                        start=True,
                        stop=True,
                    )
                    if ch == 0:
                        nc.vector.tensor_scalar_mul(
                            c2q_sb[:, ch * 512 : (ch + 1) * 512],
                            cp,
                            rinvs[:, ci : ci + 1],
                        )
                    else:
                        nc.scalar.mul(
                            c2q_sb[:, ch * 512 : (ch + 1) * 512],
                            cp,
                            rinvs[:, ci : ci + 1],
                        )
                nc.sync.dma_start(
                    out=c2q_ext[b, ci * 128 : (ci + 1) * 128], in_=c2q_sb
                )

            # ---- q2c = (z @ C) / sum(z), chunks packed in col groups ----
            q2cpp = q2c_psum.tile([64, 512], F32, tag="q2cp", name=f"q2cpp{b}")
            for ci in range(NCT):
                for ch in range(2):
                    nc.tensor.matmul(
                        q2cpp[32 * ch : 32 * ch + 1, :],
                        zcols[:, ci : ci + 1],
                        cn[ci][:, ch * 512 : (ch + 1) * 512],
                        start=(ci == 0),
                        stop=(ci == NCT - 1),
                        tile_position=(0, 32 * ch),
                    )
            q2cp = [q2cpp[0:1, :], q2cpp[32:33, :]]
            zsp = tp_psum.tile([1, NCT], F32, tag="tpose", name=f"zsp{b}")
            nc.tensor.matmul(zsp, ones_col, zcols, start=True, stop=True)
            zsum = small.tile([1, 1], F32, tag="zsum")
            nc.vector.reduce_sum(zsum, zsp, axis=AX.X)
            zrinv = small.tile([1, 1], F32, tag="zrinv")
            nc.vector.reciprocal(zrinv, zsum)
            q2c_sb = out_pool.tile([1, D], F32, tag="q2c_sb", name=f"q2csb{b}")
            for ch in range(2):
                nc.vector.tensor_scalar_mul(
                    q2c_sb[:, ch * 512 : (ch + 1) * 512], q2cp[ch], zrinv
                )
            nc.sync.dma_start(out=q2c_ext[b : b + 1, :], in_=q2c_sb)

    nc.compile()
    return nc


def _get_nc():
    global _NC_CACHE
    if _NC_CACHE is None:
        _NC_CACHE = build_kernel()
    return _NC_CACHE


def kernel(context_features, question_features, w, _trace=False):
    nc = _get_nc()
    context_features = np.ascontiguousarray(context_features, dtype=np.float32)
    question_features = np.ascontiguousarray(question_features, dtype=np.float32)
    w = np.ascontiguousarray(w, dtype=np.float32)
    in_maps = []
    for core in range(NCORES):
        b0 = core * BPC
        in_maps.append(
            {
                "ctx": context_features[b0 : b0 + BPC],
                "q": question_features[b0 : b0 + BPC],
                "w": w,
            }
        )
    res = run_bass_kernel_spmd(
        nc, in_maps, core_ids=list(range(NCORES)), trace=_trace
    )
    c2q = np.concatenate([res.results[i]["c2q"] for i in range(NCORES)], axis=0)
    q2c_vec = np.concatenate([res.results[i]["q2c"] for i in range(NCORES)], axis=0)
    q2c = np.broadcast_to(q2c_vec[:, None, :], (B, LC, D))
    if _trace:
        kernel.last_exec_time_ns = res.exec_time_ns
    return (c2q, q2c)


# revision 15
# speedup vs baseline: 1.2167x; 1.1609x over previous
"""BiDAF attention-flow kernel for one TRN2 chip (8 NeuronCores).

Reference computation (per batch b):
    w1, w2, w3 = w[:D], w[D:2D], w[2D:]
    sim[c,q] = w1.C_c + w2.Q_q + w3.(C_c*Q_q)          # trilinear similarity
    c2q = softmax_q(sim) @ Q                            # [Lc, D]
    batt = softmax_c(max_q sim)                         # [Lc]
    q2c  = batt @ C, broadcast over Lc                  # [Lc, D]
    returns (c2q, q2c_broadcast)

Sharding: pure data parallel — batch 32 split 4-per-core over 8 cores, w
replicated.  No collectives.

Device algorithm per core (4 batches), bf16 matmul inputs / f32 PSUM:
  - C, Q loaded with f32->bf16 cast during SWDGE DMA.
  - sim kept in [q, c] layout: lhsT = (w3*Q)^T tile (stationary, reused
    across the c stream), rhs = C^T streamed N=512.  PE transposes C/Q
    (bf16, grouped 8 per one [128,1024]-bf16 PSUM bank).
  - s1[c] = C@w1 and s2[q] = Q@w2 via DVE fused mul+reduce against
    partition-broadcast copies of w1/w2 (results land as columns).
  - softmax over q never needs s1 (constant per column in this layout):
    ET = exp(sim + s2) via ACT with per-partition bias; column sums/max of
    ET over q come from PE-transposed ET tiles reduced straight out of
    PSUM.
  - c2q = (ET/rsum)^T @ Q with ET used directly as lhsT; 1/rsum applied
    per-partition during PSUM evacuation.
  - z = max_q(ET) * exp(s1)  (exp is monotonic), q2c = (z @ C)/sum(z).
q2c is returned [B, D] and broadcast to [B, Lc, D] on host (replication =
part of unsharding).
"""

import sys

for _p in ("/opt/trn_rl_repo", "/root/.axon_site/_ro/trn_rl_repo"):
    if _p not in sys.path:
        sys.path.append(_p)

from contextlib import ExitStack

import numpy as np

import concourse.bacc as bacc
import concourse.bass as bass
import concourse.tile as tile
from concourse import mybir
from concourse.bass_utils import run_bass_kernel_spmd
from concourse.masks import make_identity

if __import__("os").environ.get("BASS_LDW_OPT") == "1":
    import concourse.bass_utils as _bu

    _orig_run_command = _bu.run_command

    def _patched_run_command(argv, **kw):
        argv = [
            "--enable-ldw-opt=true" if a == "--enable-ldw-opt=false" else a
            for a in argv
        ]
        return _orig_run_command(argv, **kw)

    _bu.run_command = _patched_run_command

F32 = mybir.dt.float32
BF16 = mybir.dt.bfloat16
AF = mybir.ActivationFunctionType
AX = mybir.AxisListType
ALU = mybir.AluOpType

B, LC, LQ, D = 32, 1024, 128, 1024
NCORES = 8
BPC = B // NCORES  # batches per core
NCT = LC // 128  # c-tiles per batch
NDT = D // 128  # d-tiles

_NC_CACHE = None


def build_kernel():
    nc = bacc.Bacc("TRN2", target_bir_lowering=False, debug=False, num_devices=NCORES)
    ctx_ext = nc.dram_tensor("ctx", [BPC, LC, D], F32, kind="ExternalInput").ap()
    q_ext = nc.dram_tensor("q", [BPC, LQ, D], F32, kind="ExternalInput").ap()
    w_ext = nc.dram_tensor("w", [3 * D], F32, kind="ExternalInput").ap()
    c2q_ext = nc.dram_tensor("c2q", [BPC, LC, D], F32, kind="ExternalOutput").ap()
    q2c_ext = nc.dram_tensor("q2c", [BPC, D], F32, kind="ExternalOutput").ap()

    with tile.TileContext(nc) as tc, ExitStack() as ctx:
        consts = ctx.enter_context(tc.tile_pool(name="consts", bufs=1))
        cn_pool = ctx.enter_context(tc.tile_pool(name="cn", bufs=2 * NCT))
        ct_pool = ctx.enter_context(tc.tile_pool(name="ct", bufs=2 * NDT))
        qn_pool = ctx.enter_context(tc.tile_pool(name="qn", bufs=2))
        et_pool = ctx.enter_context(tc.tile_pool(name="et", bufs=4))
        out_pool = ctx.enter_context(tc.tile_pool(name="outs", bufs=4))
        small = ctx.enter_context(tc.tile_pool(name="small", bufs=4))
        # PSUM: 8 banks.  tags: tpose(2) + simp(2) + c2qp(2) + q2cp(2)
        tp_psum = ctx.enter_context(tc.tile_pool(name="tpose", bufs=2, space="PSUM"))
        sim_psum = ctx.enter_context(tc.tile_pool(name="simp", bufs=2, space="PSUM"))
        c2q_psum = ctx.enter_context(tc.tile_pool(name="c2qp", bufs=2, space="PSUM"))
        q2c_psum = ctx.enter_context(tc.tile_pool(name="q2cp", bufs=2, space="PSUM"))

        # ---- constants ----
        ident_bf = consts.tile([128, 128], BF16)
        make_identity(nc, ident_bf)
        ident_f32 = consts.tile([128, 128], F32)
        make_identity(nc, ident_f32)
        ones_col = consts.tile([128, 1], BF16)
        nc.vector.memset(ones_col, 1.0)

        # w1, w2, w3 as per-partition columns [128, NDT]
        wsb = [
            consts.tile([NDT, 128], F32, tag=f"wsb{i}", name=f"wsb{i}")
            for i in range(3)
        ]
        for i in range(3):
            nc.sync.dma_start(
                out=wsb[i],
                in_=w_ext[i * D : (i + 1) * D].rearrange("(a b) -> a b", b=128),
            )
        wp = []
        for i in range(3):
            p = tp_psum.tile([128, NDT], F32, tag="tpose", name=f"wp{i}")
            nc.tensor.transpose(p, wsb[i], ident_f32[:NDT, :NDT])
            wp.append(p)
        w1cols = consts.tile([128, NDT], BF16)
        nc.vector.tensor_copy(w1cols, wp[0])
        w2cols = consts.tile([128, NDT], F32)
        nc.vector.tensor_copy(w2cols, wp[1])
        w3cols = consts.tile([128, NDT], F32)
        nc.vector.tensor_copy(w3cols, wp[2])
        w3inv = consts.tile([128, NDT], F32)
        nc.vector.reciprocal(w3inv, w3cols)
        w23cols = consts.tile([128, NDT], BF16)
        nc.vector.tensor_mul(w23cols, w2cols, w3inv)

        evac = 0  # DVE/ACT alternation counter

        for b in range(BPC):
            # ---- loads (cast f32 -> bf16 during DMA) ----
            qn = qn_pool.tile([LQ, D], BF16, tag="qn")
            nc.gpsimd.dma_start(out=qn, in_=q_ext[b])
            cn = []
            for ci in range(NCT):
                t = cn_pool.tile([128, D], BF16, tag="cn", name=f"cn{b}_{ci}")
                nc.gpsimd.dma_start(out=t, in_=ctx_ext[b, ci * 128 : (ci + 1) * 128])
                cn.append(t)

            # ---- Q transpose, scaled by w3:  qt3[d, q] = w3[d] * Q[q, d]^T ----
            qtp = tp_psum.tile([128, D], BF16, tag="tpose")
            for dt in range(NDT):
                nc.tensor.transpose(
                    qtp[:, dt * 128 : (dt + 1) * 128],
                    qn[:, dt * 128 : (dt + 1) * 128],
                    ident_bf,
                )
            qt3 = qn_pool.tile([128, D], BF16, tag="qt3")
            for dt in range(NDT):
                nc.vector.tensor_scalar_mul(
                    qt3[:, dt * 128 : (dt + 1) * 128],
                    qtp[:, dt * 128 : (dt + 1) * 128],
                    w3cols[:, dt : dt + 1],
                )

            # ---- C transpose interleaved with sim/s1/s2 matmuls so the PE
            # alternates transpose and matmul work (keeps HAM warm) ----
            ctb = []
            simp = []
            for g in range(2):
                sp = sim_psum.tile([128, 512], F32, tag="simp", name=f"simp{b}_{g}")
                simp.append(sp)
            s2p = q2c_psum.tile([128, 1], F32, tag="q2cp", name=f"s2p{b}")
            # s1row chunks packed into PSUM column groups 0/32: the two M=1
            # matmuls run concurrently on the PE via tile_position
            s1rp = q2c_psum.tile([64, 512], F32, tag="q2cp", name=f"s1rp{b}")
            for dt in range(NDT):
                ctp = tp_psum.tile([128, LC], BF16, tag="tpose", name=f"ctp{b}_{dt}")
                for ci in range(NCT):
                    nc.tensor.transpose(
                        ctp[:, ci * 128 : (ci + 1) * 128],
                        cn[ci][:, dt * 128 : (dt + 1) * 128],
                        ident_bf,
                    )
                t = ct_pool.tile([128, LC], BF16, tag="ct", name=f"ct{b}_{dt}")
                if evac % 2 == 0:
                    nc.vector.tensor_copy(t, ctp)
                else:
                    nc.scalar.copy(t, ctp)
                evac += 1
                ctb.append(t)
            for dt in range(NDT):
                for g in range(2):
                    nc.tensor.matmul(
                        simp[g],
                        qt3[:, dt * 128 : (dt + 1) * 128],
                        ctb[dt][:, g * 512 : (g + 1) * 512],
                        start=(dt == 0),
                        stop=(dt == NDT - 1),
                    )
                nc.tensor.matmul(
                    s2p,
                    qt3[:, dt * 128 : (dt + 1) * 128],
                    w23cols[:, dt : dt + 1],
                    start=(dt == 0),
                    stop=(dt == NDT - 1),
                )
                for g in range(2):
                    nc.tensor.matmul(
                        s1rp[32 * g : 32 * g + 1, :],
                        w1cols[:, dt : dt + 1],
                        ctb[dt][:, g * 512 : (g + 1) * 512],
                        start=(dt == 0),
                        stop=(dt == NDT - 1),
                        tile_position=(0, 32 * g),
                    )
            s2c = small.tile([128, 1], F32, tag="s2c")
            nc.vector.tensor_copy(s2c, s2p)

            s1row_sb = small.tile([1, LC], F32, tag="s1row")
            for g in range(2):
                nc.vector.tensor_copy(
                    s1row_sb[:, g * 512 : (g + 1) * 512],
                    s1rp[32 * g : 32 * g + 1, :],
                )
            s1p = q2c_psum.tile([128, NCT], F32, tag="q2cp", name=f"s1p{b}")
            for ci in range(NCT):
                nc.tensor.transpose(
                    s1p[:, ci : ci + 1],
                    s1row_sb[0:1, ci * 128 : (ci + 1) * 128],
                    ident_f32[0:1, 0:1],
                )
            s1cols = small.tile([128, NCT], F32, tag="s1cols")
            nc.vector.tensor_copy(s1cols, s1p)
            es1 = small.tile([128, NCT], F32, tag="es1")
            nc.scalar.activation(es1, s1cols, AF.Exp)

            # ---- ET = exp(sim + s2)  [q, c] bf16 ----
            et = []
            for g in range(2):
                e = et_pool.tile([128, 512], BF16, tag="et", name=f"et{b}_{g}")
                nc.scalar.activation(e, simp[g], AF.Exp, bias=s2c)
                et.append(e)

            # ---- ET transposed -> column-wise max (z) and sum (rsum).
            # One fast evac to SBUF releases the PSUM bank early; the max and
            # sum then run as single 3D-AP reduces. ----
            zraw = small.tile([128, NCT], F32, tag="zraw")
            rsums = small.tile([128, NCT], F32, tag="rsums")
            etp = tp_psum.tile([128, LC], BF16, tag="tpose", name=f"etp{b}")
            for ci in range(NCT):
                nc.tensor.transpose(
                    etp[:, ci * 128 : (ci + 1) * 128],
                    et[ci // 4][:, (ci % 4) * 128 : (ci % 4 + 1) * 128],
                    ident_bf,
                )
            ets = qn_pool.tile([128, LC], BF16, tag="ets", name=f"ets{b}")
            nc.vector.tensor_copy(ets, etp)
            ets3 = ets.rearrange("p (t c) -> p t c", c=128)
            nc.vector.reduce_max(zraw, ets3, axis=AX.X)
            nc.vector.reduce_sum(rsums, ets3, axis=AX.X)

            rinvs = small.tile([128, NCT], F32, tag="rinvs")
            nc.vector.reciprocal(rinvs, rsums)
            zcols = small.tile([128, NCT], BF16, tag="zcols")
            nc.vector.tensor_mul(zcols, zraw, es1)

            # ---- c2q matmuls per c-tile ----
            for ci in range(NCT):
                lhs = et[ci // 4][:, (ci % 4) * 128 : (ci % 4 + 1) * 128]
                c2q_sb = out_pool.tile(
                    [128, D], F32, tag="c2q_sb", name=f"c2qsb{b}_{ci}"
                )
                for ch in range(2):
                    cp = c2q_psum.tile(
                        [128, 512], F32, tag="c2qp", name=f"cp{b}_{ci}_{ch}"
                    )
                    nc.tensor.matmul(
                        cp,
                        lhs,
                        qn[:, ch * 512 : (ch + 1) * 512],
                        start=True,
                        stop=True,
                    )
                    if ch == 0:
                        nc.vector.tensor_scalar_mul(
                            c2q_sb[:, ch * 512 : (ch + 1) * 512],
                            cp,
                            rinvs[:, ci : ci + 1],
                        )
                    else:
                        nc.scalar.mul(
                            c2q_sb[:, ch * 512 : (ch + 1) * 512],
                            cp,
                            rinvs[:, ci : ci + 1],
                        )
                nc.sync.dma_start(
                    out=c2q_ext[b, ci * 128 : (ci + 1) * 128], in_=c2q_sb
                )

            # ---- q2c = (z @ C) / sum(z), chunks packed in col groups ----
            q2cpp = q2c_psum.tile([64, 512], F32, tag="q2cp", name=f"q2cpp{b}")
            for ci in range(NCT):
                for ch in range(2):
                    nc.tensor.matmul(
                        q2cpp[32 * ch : 32 * ch + 1, :],
                        zcols[:, ci : ci + 1],
                        cn[ci][:, ch * 512 : (ch + 1) * 512],
                        start=(ci == 0),
                        stop=(ci == NCT - 1),
                        tile_position=(0, 32 * ch),
                    )
            q2cp = [q2cpp[0:1, :], q2cpp[32:33, :]]
            zsp = tp_psum.tile([1, NCT], F32, tag="tpose", name=f"zsp{b}")
            nc.tensor.matmul(zsp, ones_col, zcols, start=True, stop=True)
            zsum = small.tile([1, 1], F32, tag="zsum")
            nc.vector.reduce_sum(zsum, zsp, axis=AX.X)
            zrinv = small.tile([1, 1], F32, tag="zrinv")
            nc.vector.reciprocal(zrinv, zsum)
            q2c_sb = out_pool.tile([1, D], F32, tag="q2c_sb", name=f"q2csb{b}")
            for ch in range(2):
                nc.vector.tensor_scalar_mul(
                    q2c_sb[:, ch * 512 : (ch + 1) * 512], q2cp[ch], zrinv
                )
            nc.sync.dma_start(out=q2c_ext[b : b + 1, :], in_=q2c_sb)

    nc.compile()
    return nc


def _get_nc():
    global _NC_CACHE
    if _NC_CACHE is None:
        _NC_CACHE = build_kernel()
    return _NC_CACHE


def kernel(context_features, question_features, w, _trace=False):
    nc = _get_nc()
    context_features = np.ascontiguousarray(context_features, dtype=np.float32)
    question_features = np.ascontiguousarray(question_features, dtype=np.float32)
    w = np.ascontiguousarray(w, dtype=np.float32)
    in_maps = []
    for core in range(NCORES):
        b0 = core * BPC
        in_maps.append(
            {
                "ctx": context_features[b0 : b0 + BPC],
                "q": question_features[b0 : b0 + BPC],
                "w": w,
            }
        )
    res = run_bass_kernel_spmd(
        nc, in_maps, core_ids=list(range(NCORES)), trace=_trace
    )
    c2q = np.concatenate([res.results[i]["c2q"] for i in range(NCORES)], axis=0)
    q2c_vec = np.concatenate([res.results[i]["q2c"] for i in range(NCORES)], axis=0)
    q2c = np.broadcast_to(q2c_vec[:, None, :], (B, LC, D))
    if _trace:
        kernel.last_exec_time_ns = res.exec_time_ns
    return (c2q, q2c)


# revision 17
# speedup vs baseline: 1.4514x; 1.1929x over previous
"""BiDAF attention-flow kernel for one TRN2 chip (8 NeuronCores).

Reference computation (per batch b):
    w1, w2, w3 = w[:D], w[D:2D], w[2D:]
    sim[c,q] = w1.C_c + w2.Q_q + w3.(C_c*Q_q)          # trilinear similarity
    c2q = softmax_q(sim) @ Q                            # [Lc, D]
    batt = softmax_c(max_q sim)                         # [Lc]
    q2c  = batt @ C, broadcast over Lc                  # [Lc, D]
    returns (c2q, q2c_broadcast)

Sharding: pure data parallel — batch 32 split 4-per-core over 8 cores, w
replicated.  No collectives.

Device algorithm per core (4 batches), bf16 matmul inputs / f32 PSUM:
  - C, Q loaded with f32->bf16 cast during SWDGE DMA.
  - sim kept in [q, c] layout: lhsT = (w3*Q)^T tile (stationary, reused
    across the c stream), rhs = C^T streamed N=512.  PE transposes C/Q
    (bf16, grouped 8 per one [128,1024]-bf16 PSUM bank).
  - s1[c] = C@w1 and s2[q] = Q@w2 via DVE fused mul+reduce against
    partition-broadcast copies of w1/w2 (results land as columns).
  - softmax over q never needs s1 (constant per column in this layout):
    ET = exp(sim + s2) via ACT with per-partition bias; column sums/max of
    ET over q come from PE-transposed ET tiles reduced straight out of
    PSUM.
  - c2q = (ET/rsum)^T @ Q with ET used directly as lhsT; 1/rsum applied
    per-partition during PSUM evacuation.
  - z = max_q(ET) * exp(s1)  (exp is monotonic), q2c = (z @ C)/sum(z).
q2c is returned [B, D] and broadcast to [B, Lc, D] on host (replication =
part of unsharding).
"""

import sys

for _p in ("/opt/trn_rl_repo", "/root/.axon_site/_ro/trn_rl_repo"):
    if _p not in sys.path:
        sys.path.append(_p)

from contextlib import ExitStack

import numpy as np

import concourse.bacc as bacc
import concourse.bass as bass
import concourse.tile as tile
from concourse import mybir
from concourse.bass_utils import run_bass_kernel_spmd
from concourse.masks import make_identity

if __import__("os").environ.get("BASS_LDW_OPT") == "1":
    import concourse.bass_utils as _bu

    _orig_run_command = _bu.run_command

    def _patched_run_command(argv, **kw):
        argv = [
            "--enable-ldw-opt=true" if a == "--enable-ldw-opt=false" else a
            for a in argv
        ]
        return _orig_run_command(argv, **kw)

    _bu.run_command = _patched_run_command

F32 = mybir.dt.float32
BF16 = mybir.dt.bfloat16
AF = mybir.ActivationFunctionType
AX = mybir.AxisListType
ALU = mybir.AluOpType

B, LC, LQ, D = 32, 1024, 128, 1024
NCORES = 8
BPC = B // NCORES  # batches per core
NCT = LC // 128  # c-tiles per batch
NDT = D // 128  # d-tiles

_NC_CACHE = None


def build_kernel():
    nc = bacc.Bacc("TRN2", target_bir_lowering=False, debug=False, num_devices=NCORES)
    ctx_ext = nc.dram_tensor("ctx", [BPC, LC, D], F32, kind="ExternalInput").ap()
    q_ext = nc.dram_tensor("q", [BPC, LQ, D], F32, kind="ExternalInput").ap()
    w_ext = nc.dram_tensor("w", [3 * D], F32, kind="ExternalInput").ap()
    c2q_ext = nc.dram_tensor("c2q", [BPC, LC, D], F32, kind="ExternalOutput").ap()
    q2c_ext = nc.dram_tensor("q2c", [BPC, D], F32, kind="ExternalOutput").ap()

    with tile.TileContext(nc) as tc, ExitStack() as ctx:
        consts = ctx.enter_context(tc.tile_pool(name="consts", bufs=1))
        cn_pool = ctx.enter_context(tc.tile_pool(name="cn", bufs=2 * NCT))
        ct_pool = ctx.enter_context(tc.tile_pool(name="ct", bufs=2 * NDT))
        qn_pool = ctx.enter_context(tc.tile_pool(name="qn", bufs=2))
        et_pool = ctx.enter_context(tc.tile_pool(name="et", bufs=4))
        out_pool = ctx.enter_context(tc.tile_pool(name="outs", bufs=4))
        small = ctx.enter_context(tc.tile_pool(name="small", bufs=4))
        # PSUM: 8 banks.  tags: tpose(2) + simp(2) + work(4)
        tp_psum = ctx.enter_context(tc.tile_pool(name="tpose", bufs=2, space="PSUM"))
        sim_psum = ctx.enter_context(tc.tile_pool(name="simp", bufs=2, space="PSUM"))
        work_psum = ctx.enter_context(tc.tile_pool(name="work", bufs=4, space="PSUM"))

        # ---- constants ----
        ident_bf = consts.tile([128, 128], BF16)
        make_identity(nc, ident_bf)
        ident_f32 = consts.tile([128, 128], F32)
        make_identity(nc, ident_f32)
        ones_col = consts.tile([128, 1], BF16)
        nc.vector.memset(ones_col, 1.0)

        # w1, w2, w3 as per-partition columns [128, NDT]
        wsb = [
            consts.tile([NDT, 128], F32, tag=f"wsb{i}", name=f"wsb{i}")
            for i in range(3)
        ]
        for i in range(3):
            nc.sync.dma_start(
                out=wsb[i],
                in_=w_ext[i * D : (i + 1) * D].rearrange("(a b) -> a b", b=128),
            )
        wp = []
        for i in range(3):
            p = tp_psum.tile([128, NDT], F32, tag="tpose", name=f"wp{i}")
            nc.tensor.transpose(p, wsb[i], ident_f32[:NDT, :NDT])
            wp.append(p)
        w1cols = consts.tile([128, NDT], BF16)
        nc.vector.tensor_copy(w1cols, wp[0])
        w2cols = consts.tile([128, NDT], F32)
        nc.vector.tensor_copy(w2cols, wp[1])
        w3cols = consts.tile([128, NDT], F32)
        nc.vector.tensor_copy(w3cols, wp[2])
        w3inv = consts.tile([128, NDT], F32)
        nc.vector.reciprocal(w3inv, w3cols)
        w23cols = consts.tile([128, NDT], BF16)
        nc.vector.tensor_mul(w23cols, w2cols, w3inv)

        evac = 0  # DVE/ACT alternation counter

        for b in range(BPC):
            # ---- loads (cast f32 -> bf16 during DMA) ----
            qn = qn_pool.tile([LQ, D], BF16, tag="qn")
            nc.gpsimd.dma_start(out=qn, in_=q_ext[b])
            cn = []
            for ci in range(NCT):
                t = cn_pool.tile([128, D], BF16, tag="cn", name=f"cn{b}_{ci}")
                nc.gpsimd.dma_start(out=t, in_=ctx_ext[b, ci * 128 : (ci + 1) * 128])
                cn.append(t)

            # ---- Q transpose, scaled by w3:  qt3[d, q] = w3[d] * Q[q, d]^T ----
            qtp = tp_psum.tile([128, D], BF16, tag="tpose")
            for dt in range(NDT):
                nc.tensor.transpose(
                    qtp[:, dt * 128 : (dt + 1) * 128],
                    qn[:, dt * 128 : (dt + 1) * 128],
                    ident_bf,
                )
            qt3 = qn_pool.tile([128, D], BF16, tag="qt3")
            for dt in range(NDT):
                nc.vector.tensor_scalar_mul(
                    qt3[:, dt * 128 : (dt + 1) * 128],
                    qtp[:, dt * 128 : (dt + 1) * 128],
                    w3cols[:, dt : dt + 1],
                )

            # ---- C transpose interleaved with sim/s1/s2 matmuls so the PE
            # alternates transpose and matmul work (keeps HAM warm) ----
            ctb = []
            simp = []
            for g in range(2):
                sp = sim_psum.tile([128, 512], F32, tag="simp", name=f"simp{b}_{g}")
                simp.append(sp)
            s2p = work_psum.tile([128, 1], F32, tag="work", name=f"s2p{b}")
            # s1row chunks packed into PSUM column groups 0/32: the two M=1
            # matmuls run concurrently on the PE via tile_position
            s1rp = work_psum.tile([64, 512], F32, tag="work", name=f"s1rp{b}")
            for dt in range(NDT):
                ctp = tp_psum.tile([128, LC], BF16, tag="tpose", name=f"ctp{b}_{dt}")
                for ci in range(NCT):
                    nc.tensor.transpose(
                        ctp[:, ci * 128 : (ci + 1) * 128],
                        cn[ci][:, dt * 128 : (dt + 1) * 128],
                        ident_bf,
                    )
                t = ct_pool.tile([128, LC], BF16, tag="ct", name=f"ct{b}_{dt}")
                if evac % 2 == 0:
                    nc.vector.tensor_copy(t, ctp)
                else:
                    nc.scalar.copy(t, ctp)
                evac += 1
                ctb.append(t)
            for dt in range(NDT):
                for g in range(2):
                    nc.tensor.matmul(
                        simp[g],
                        qt3[:, dt * 128 : (dt + 1) * 128],
                        ctb[dt][:, g * 512 : (g + 1) * 512],
                        start=(dt == 0),
                        stop=(dt == NDT - 1),
                    )
                nc.tensor.matmul(
                    s2p,
                    qt3[:, dt * 128 : (dt + 1) * 128],
                    w23cols[:, dt : dt + 1],
                    start=(dt == 0),
                    stop=(dt == NDT - 1),
                )
                for g in range(2):
                    nc.tensor.matmul(
                        s1rp[32 * g : 32 * g + 1, :],
                        w1cols[:, dt : dt + 1],
                        ctb[dt][:, g * 512 : (g + 1) * 512],
                        start=(dt == 0),
                        stop=(dt == NDT - 1),
                        tile_position=(0, 32 * g),
                    )
            s2c = small.tile([128, 1], F32, tag="s2c")
            nc.vector.tensor_copy(s2c, s2p)

            s1row_sb = small.tile([1, LC], F32, tag="s1row")
            for g in range(2):
                nc.vector.tensor_copy(
                    s1row_sb[:, g * 512 : (g + 1) * 512],
                    s1rp[32 * g : 32 * g + 1, :],
                )
            s1p = work_psum.tile([128, NCT], F32, tag="work", name=f"s1p{b}")
            for ci in range(NCT):
                nc.tensor.transpose(
                    s1p[:, ci : ci + 1],
                    s1row_sb[0:1, ci * 128 : (ci + 1) * 128],
                    ident_f32[0:1, 0:1],
                )
            s1cols = small.tile([128, NCT], F32, tag="s1cols")
            nc.vector.tensor_copy(s1cols, s1p)
            es1 = small.tile([128, NCT], F32, tag="es1")
            nc.scalar.activation(es1, s1cols, AF.Exp)

            # ---- ET = exp(sim + s2)  [q, c] bf16 ----
            et = []
            for g in range(2):
                e = et_pool.tile([128, 512], BF16, tag="et", name=f"et{b}_{g}")
                nc.scalar.activation(e, simp[g], AF.Exp, bias=s2c)
                et.append(e)

            # ---- ET transposed -> column-wise max (z) and sum (rsum).
            # One fast evac to SBUF releases the PSUM bank early; the max and
            # sum then run as single 3D-AP reduces. ----
            zraw = small.tile([128, NCT], F32, tag="zraw")
            rsums = small.tile([128, NCT], F32, tag="rsums")
            etp = tp_psum.tile([128, LC], BF16, tag="tpose", name=f"etp{b}")
            for ci in range(NCT):
                nc.tensor.transpose(
                    etp[:, ci * 128 : (ci + 1) * 128],
                    et[ci // 4][:, (ci % 4) * 128 : (ci % 4 + 1) * 128],
                    ident_bf,
                )
            ets = qn_pool.tile([128, LC], BF16, tag="ets", name=f"ets{b}")
            nc.vector.tensor_copy(ets, etp)
            ets3 = ets.rearrange("p (t c) -> p t c", c=128)
            nc.vector.reduce_max(zraw, ets3, axis=AX.X)
            nc.vector.reduce_sum(rsums, ets3, axis=AX.X)

            rinvs = small.tile([128, NCT], F32, tag="rinvs")
            nc.vector.reciprocal(rinvs, rsums)
            zcols = small.tile([128, NCT], BF16, tag="zcols")
            nc.vector.tensor_mul(zcols, zraw, es1)

            # ---- c2q matmuls per c-tile ----
            for ci in range(NCT):
                lhs = et[ci // 4][:, (ci % 4) * 128 : (ci % 4 + 1) * 128]
                c2q_sb = out_pool.tile(
                    [128, D], F32, tag="c2q_sb", name=f"c2qsb{b}_{ci}"
                )
                for ch in range(2):
                    cp = work_psum.tile(
                        [128, 512], F32, tag="work", name=f"cp{b}_{ci}_{ch}"
                    )
                    nc.tensor.matmul(
                        cp,
                        lhs,
                        qn[:, ch * 512 : (ch + 1) * 512],
                        start=True,
                        stop=True,
                    )
                    if ch == 0:
                        nc.vector.tensor_scalar_mul(
                            c2q_sb[:, ch * 512 : (ch + 1) * 512],
                            cp,
                            rinvs[:, ci : ci + 1],
                        )
                    else:
                        nc.scalar.mul(
                            c2q_sb[:, ch * 512 : (ch + 1) * 512],
                            cp,
                            rinvs[:, ci : ci + 1],
                        )
                nc.sync.dma_start(
                    out=c2q_ext[b, ci * 128 : (ci + 1) * 128], in_=c2q_sb
                )

            # ---- q2c = (z @ C) / sum(z), chunks packed in col groups ----
            q2cpp = work_psum.tile([64, 512], F32, tag="work", name=f"q2cpp{b}")
            for ci in range(NCT):
                for ch in range(2):
                    nc.tensor.matmul(
                        q2cpp[32 * ch : 32 * ch + 1, :],
                        zcols[:, ci : ci + 1],
                        cn[ci][:, ch * 512 : (ch + 1) * 512],
                        start=(ci == 0),
                        stop=(ci == NCT - 1),
                        tile_position=(0, 32 * ch),
                    )
            q2cp = [q2cpp[0:1, :], q2cpp[32:33, :]]
            zsp = tp_psum.tile([1, NCT], F32, tag="tpose", name=f"zsp{b}")
            nc.tensor.matmul(zsp, ones_col, zcols, start=True, stop=True)
            zsum = small.tile([1, 1], F32, tag="zsum")
            nc.vector.reduce_sum(zsum, zsp, axis=AX.X)
            zrinv = small.tile([1, 1], F32, tag="zrinv")
            nc.vector.reciprocal(zrinv, zsum)
            q2c_sb = out_pool.tile([1, D], F32, tag="q2c_sb", name=f"q2csb{b}")
            for ch in range(2):
                nc.vector.tensor_scalar_mul(
                    q2c_sb[:, ch * 512 : (ch + 1) * 512], q2cp[ch], zrinv
                )
            nc.sync.dma_start(out=q2c_ext[b : b + 1, :], in_=q2c_sb)

    nc.compile()
    return nc


def _get_nc():
    global _NC_CACHE
    if _NC_CACHE is None:
        _NC_CACHE = build_kernel()
    return _NC_CACHE


def kernel(context_features, question_features, w, _trace=False):
    nc = _get_nc()
    context_features = np.ascontiguousarray(context_features, dtype=np.float32)
    question_features = np.ascontiguousarray(question_features, dtype=np.float32)
    w = np.ascontiguousarray(w, dtype=np.float32)
    in_maps = []
    for core in range(NCORES):
        b0 = core * BPC
        in_maps.append(
            {
                "ctx": context_features[b0 : b0 + BPC],
                "q": question_features[b0 : b0 + BPC],
                "w": w,
            }
        )
    res = run_bass_kernel_spmd(
        nc, in_maps, core_ids=list(range(NCORES)), trace=_trace
    )
    c2q = np.concatenate([res.results[i]["c2q"] for i in range(NCORES)], axis=0)
    q2c_vec = np.concatenate([res.results[i]["q2c"] for i in range(NCORES)], axis=0)
    q2c = np.broadcast_to(q2c_vec[:, None, :], (B, LC, D))
    if _trace:
        kernel.last_exec_time_ns = res.exec_time_ns
    return (c2q, q2c)
